# revision 39
# baseline (speedup 1.0000x reference)
"""Bass/Trainium2 kernel for nn_ExpressionEncoder (conv-QKV attention + BN).

Data-parallel over batch: 8 images -> 8 NeuronCores, one image per core.

v2: fp8 (TRN e4m3) DoubleRow matmuls for the convs and the S^T (logits)
matmul -- K=256 contraction per instruction at ~1.8x the bf16 rate (the
PE runs ~2.0 GHz under full-chip load while LDWEIGHTS stays on the
1.2 GHz NX clock, so DoubleRow's 2-wide rows win big). Numerics
validated offline vs the fp32 reference: l2 ~4e-3 (tolerance 2e-2).

Per-core pipeline (everything on-chip between input DMA and output DMA):
  1. Host packs x twice: fp8 xpad [128, 2, 4368] (channel-pair conv
     input, flat padded 66x66 rows) and bf16 x [256, 4096] (residual).
     Weights are packed as DoubleRow pairs: wkv [128, 9, 2, 512],
     wq [128, 2, 256].
  2. KV conv: per 128-channel output group and 7-row block, 9
     accumulating DoubleRow matmuls over contiguous flat windows
     (seam columns between rows compute garbage and are skipped by the
     relu's strided read). K -> kt fp8 [128, 2, 4096]; V -> fp32 vt,
     PE-transposed into V' [j, 257] bf16 with a trailing ones column
     (softmax denominator falls out of the A@V matmul for free).
     Q (1x1 conv) -> qt fp8 [128, 2, 4096] the same way.
  3. Attention, software-pipelined per 512-query block i: the 32
     S^T DoubleRow matmuls of block i are interleaved with the 128
     bf16 A@V matmuls of block i-1, so the PE stays busy while ScalarE
     exps block i (exp output is fp8 with exp(x/16 - 9) -- scale
     cancels in the softmax ratio; max logit ~12.8 so no overflow at
     the TRN e4m3 +-240 clip). A@V + ones column -> normalize ->
     PE-transpose back to [d, i] -> residual add -> y fp32; BN partial
     sums ride the same DVE op via accum_out.
  4. AllReduce (8 cores) of per-channel [sum(y), sum(y^2)] -> scale a,
     bias b -> out = a*y + b in bf16 (chunks alternate ScalarE/VectorE,
     chunk DMAs split across both HWDGE queue groups; host upcasts to
     fp32). A warmup AllReduce runs during the conv so the real one
     doesn't pay cold ALGO_MESH setup.

Scheduling notes (measured on HW): keep the PE transposes exactly at
their natural drain points -- deferring them to fill visible PE gaps
breaks the LDWEIGHTS background-buffer pipelining and inflates every
matmul in the stream by ~10%. The attention phase is bounded by
LDWEIGHTS column throughput (NX at 1.2 GHz): A@V reloads a 256-col
stationary per 257 moving columns, which is unavoidable while the
softmax denominator rides the V' ones column (operand-flipping A@V
would need a cross-partition reduction for Z instead).
"""

import os
import sys

for _p in ("/opt/trn_rl_repo", os.path.expanduser("~/.axon_site/_ro/trn_rl_repo")):
    if os.path.isdir(_p) and _p not in sys.path:
        sys.path.append(_p)

import numpy as np

import concourse.bass as bass
import concourse.tile as tile
from concourse import bacc, mybir
from concourse.bass_utils import run_bass_kernel_spmd
from concourse.masks import make_identity

dt = mybir.dt
F32 = dt.float32
BF16 = dt.bfloat16
FP8 = dt.float8e4

N_CORES = 8
C = 256        # channels (= dm)
HW = 64        # spatial side
N = HW * HW    # tokens per image
PW = HW + 2    # padded side
FLAT = PW * PW          # 4356
FLATP = 4368            # padded to a 16-multiple for DoubleRow strides
IBLK = 512
N_IBLK = N // IBLK      # 8
N_JT = N // 128         # 32
BN_EPS = 1e-5
INV_SQRT_DM = 1.0 / 16.0
EXP_BIAS = -9.0         # exp(sim/16 - 9): keeps fp8 et under the 240 clip
# conv row blocks: (first output row, rows). 7-row blocks have a 460-wide
# flat output window; the final 8 rows go in two 4-row blocks (262-wide)
# to keep the matmul free dim >= 256.
BLOCKS = [(0, 7), (7, 7), (14, 7), (21, 7), (28, 7), (35, 7), (42, 7),
          (49, 7), (56, 4), (60, 4)]
DR = mybir.MatmulPerfMode.DoubleRow


def build_program(n_cores=N_CORES, replica_groups=None):
    if replica_groups is None:
        replica_groups = [list(range(n_cores))]
    nc = bacc.Bacc(
        "TRN2", target_bir_lowering=False, debug=False, num_devices=n_cores
    )
    xpad_d = nc.dram_tensor("xpad", [128, 2, FLATP], FP8, kind="ExternalInput")
    xres_d = nc.dram_tensor("xres", [C, N], BF16, kind="ExternalInput")
    wkv_d = nc.dram_tensor("wkv", [128, 4, 9, 2, 128], FP8, kind="ExternalInput")
    wq_d = nc.dram_tensor("wq", [128, 2, C], FP8, kind="ExternalInput")
    smalls_d = nc.dram_tensor("smalls", [128, 10], F32, kind="ExternalInput")
    out_d = nc.dram_tensor("out", [C, N], BF16, kind="ExternalOutput")

    with tile.TileContext(nc) as tc:
        _body(tc, xpad_d, xres_d, wkv_d, wq_d, smalls_d, out_d, replica_groups)
    nc.compile()
    return nc


def _body(tc, xpad_d, xres_d, wkv_d, wq_d, smalls_d, out_d, replica_groups):
    nc = tc.nc
    from contextlib import ExitStack

    ctx = ExitStack()
    with ctx:
        const = ctx.enter_context(tc.tile_pool(name="const", bufs=1))
        et_pool = ctx.enter_context(tc.tile_pool(name="et", bufs=2))
        rn_pool = ctx.enter_context(tc.tile_pool(name="rn", bufs=2))
        sq_pool = ctx.enter_context(tc.tile_pool(name="sq", bufs=2))
        tiny = ctx.enter_context(tc.tile_pool(name="tiny", bufs=2))
        dram = ctx.enter_context(tc.tile_pool(name="dram", bufs=1, space="DRAM"))
        ps_mm = ctx.enter_context(tc.tile_pool(name="ps_mm", bufs=2, space="PSUM"))
        ps_av = ctx.enter_context(tc.tile_pool(name="ps_av", bufs=2, space="PSUM"))
        ps_tr = ctx.enter_context(tc.tile_pool(name="ps_tr", bufs=2, space="PSUM"))

        # HAM pre-warm: the PE clock-gate needs ~3.4us of sustained matmul
        # activity to go 4/8 -> 8/8; burn dummy matmuls on a zeroed tile
        # during the input-DMA gate so the first real conv matmuls run at
        # full clock
        warm_w = const.tile([128, 128], BF16)
        nc.vector.memset(warm_w[:], 0.0)
        ps_warm = ps_mm.tile([128, 1024], F32, tag="mm", name="ps_warm")
        for k in range(72):
            nc.tensor.matmul(
                ps_warm[:, 0:128],
                lhsT=warm_w[:],
                rhs=warm_w[:],
                start=(k == 0),
                stop=(k == 71),
            )

        # ---- inputs (conv inputs first -- they gate the PE start) ----
        xpad = const.tile([128, 2, FLATP], FP8)
        wkv_sb = const.tile([128, 4, 9, 2, 128], FP8)
        for s in range(2):
            nc.sync.dma_start(out=xpad[:, s, 0:594], in_=xpad_d[:, s, 0:594])
        for g in range(4):
            nc.sync.dma_start(out=wkv_sb[:, g, :, :, :], in_=wkv_d[:, g, :, :, :])
        for s in range(2):
            nc.sync.dma_start(out=xpad[:, s, 594:2184], in_=xpad_d[:, s, 594:2184])
        for s in range(2):
            nc.sync.dma_start(out=xpad[:, s, 2184:FLATP], in_=xpad_d[:, s, 2184:FLATP])
        smalls = const.tile([128, 10], F32)
        nc.sync.dma_start(out=smalls[:], in_=smalls_d[:])
        ident = const.tile([128, 128], F32)
        make_identity(nc, ident[:])
        wq_sb = const.tile([128, 2, C], FP8)
        nc.sync.dma_start(out=wq_sb[:], in_=wq_d[:])
        xres = [const.tile([128, N], BF16, name=f"xres{ct}", tag=f"xres{ct}")
                for ct in range(2)]
        for ct in range(2):
            cs = slice(ct * 128, (ct + 1) * 128)
            for hc in range(2):
                nc.sync.dma_start(
                    out=xres[ct][:, hc * 2048 : (hc + 1) * 2048],
                    in_=xres_d[cs, hc * 2048 : (hc + 1) * 2048],
                )

        # warm up the collectives firmware during the conv so the real BN
        # all-reduce doesn't pay the ~11us cold ALGO_MESH setup
        wu_sb = tiny.tile([128, 1], F32, tag="wu")
        nc.vector.memset(wu_sb[:], 0.0)
        wu_in = dram.tile([128, 1], F32)
        wu_out = dram.tile([128, 1], F32)
        nc.sync.dma_start(out=wu_in[:], in_=wu_sb[:])
        nc.gpsimd.collective_compute(
            "AllReduce",
            mybir.AluOpType.add,
            replica_groups=replica_groups,
            ins=[wu_in.opt()],
            outs=[wu_out.opt()],
        )
        wu_back = tiny.tile([128, 1], F32, tag="wub")
        nc.sync.dma_start(out=wu_back[:], in_=wu_out[:])

        # ---- persistent activations ----
        kt = const.tile([128, 2, N], FP8)
        qt = const.tile([128, 2, N], FP8)
        vt_dr = [const.tile([128, N], F32, name=f"vt{d}", tag=f"vt{d}")
                 for d in range(2)]
        # V' [j, d + ones] in fp8 DoubleRow pairs: [j_lo, t, j_hi, d] with
        # j = j_lo + 128 * (2t + j_hi); stride 272 keeps the pair step
        # 16-aligned
        vp = const.tile([128, N_JT // 2, 2, 272], FP8)
        nc.vector.memset(vp[:, :, :, 256], 1.0)
        y = [const.tile([128, N], F32, name=f"y{d}", tag=f"y{d}")
             for d in range(2)]
        ebias = const.tile([128, 1], F32)
        nc.vector.memset(ebias[:], EXP_BIAS)
        ssum = [const.tile([128, 4 * N_IBLK], F32, name=f"ssum{d}", tag=f"ssum{d}")
                for d in range(2)]
        ssq = [const.tile([128, 4 * N_IBLK], F32, name=f"ssq{d}", tag=f"ssq{d}")
               for d in range(2)]

        # ---- attention helpers (S^T block 0 overlaps the conv phase) ----
        et_tiles = {}
        av_psa = {}

        def emit_s_group(i, jp):
            pst = ps_mm.tile([128, 1024], F32, tag="mm")
            i0 = i * IBLK
            for sub in range(2):
                jt = 2 * jp + sub
                nc.tensor.matmul(
                    pst[:, sub * 512 : (sub + 1) * 512],
                    lhsT=kt[:, :, jt * 128 : (jt + 1) * 128],
                    rhs=qt[:, :, i0 : i0 + IBLK],
                    start=True,
                    stop=True,
                    perf_mode=DR,
                )
            nc.scalar.activation(
                et_tiles[i][:, 2 * jp : 2 * jp + 2, :],
                pst[:].rearrange("p (a b) -> p a b", a=2),
                mybir.ActivationFunctionType.Exp,
                bias=ebias[:],
                scale=INV_SQRT_DM,
            )

        # ---- phase B: Q/KV convs (+ V transposes as 128-j chunks land) ----
        shifts = [(kh, kw) for kh in range(3) for kw in range(3)]
        vtr_done = 0

        def emit_v_transposes(upto):
            nonlocal vtr_done
            for cch in range(vtr_done, upto):
                for dv in range(2):
                    pstr = ps_tr.tile([128, 128], F32, tag="tr")
                    nc.tensor.transpose(
                        pstr[:], vt_dr[dv][:, cch * 128 : (cch + 1) * 128], ident[:]
                    )
                    nc.vector.tensor_copy(
                        vp[:, cch // 2, cch % 2, dv * 128 : (dv + 1) * 128],
                        pstr[:],
                    )
            vtr_done = upto

        for bi, (r0, nr) in enumerate(BLOCKS):
            L = (nr - 1) * 66 + 64
            for dkvt in range(4):
                ps = ps_mm.tile([128, 1024], F32, tag="mm")
                for si, (sh, sw) in enumerate(shifts):
                    base = (r0 + sh) * 66 + sw
                    nc.tensor.matmul(
                        ps[:, 0:L],
                        lhsT=wkv_sb[:, dkvt, si, :, :],
                        rhs=xpad[:, :, base : base + L],
                        start=(si == 0),
                        stop=(si == 8),
                        perf_mode=DR,
                    )
                src = ps[:, 0 : nr * 66].rearrange("p (r w) -> p r w", w=66)[
                    :, :, 0:64
                ]
                bias = smalls[:, 2 + dkvt : 3 + dkvt]
                if dkvt < 2:
                    dst = kt[:, dkvt, r0 * 64 : (r0 + nr) * 64].rearrange(
                        "p (r w) -> p r w", w=64
                    )
                else:
                    dst = vt_dr[dkvt - 2][:, r0 * 64 : (r0 + nr) * 64].rearrange(
                        "p (r w) -> p r w", w=64
                    )
                nc.scalar.activation(
                    dst, src, mybir.ActivationFunctionType.Relu,
                    bias=bias, scale=1.0,
                )
            for dqt in range(2):
                psq = ps_mm.tile([128, 1024], F32, tag="mm")
                base = (r0 + 1) * 66 + 1
                nc.tensor.matmul(
                    psq[:, 0:L],
                    lhsT=wq_sb[:, :, dqt * 128 : (dqt + 1) * 128],
                    rhs=xpad[:, :, base : base + L],
                    start=True,
                    stop=True,
                    perf_mode=DR,
                )
                srcq = psq[:, 0 : nr * 66].rearrange("p (r w) -> p r w", w=66)[
                    :, :, 0:64
                ]
                dstq = qt[:, dqt, r0 * 64 : (r0 + nr) * 64].rearrange(
                    "p (r w) -> p r w", w=64
                )
                nc.scalar.activation(
                    dstq, srcq, mybir.ActivationFunctionType.Relu,
                    bias=smalls[:, dqt : dqt + 1], scale=1.0,
                )
            if bi == 0:
                # prime the Exp ACT table while the PE is busy with conv
                pexp = tiny.tile([128, 1], F32, tag="pexp")
                nc.scalar.activation(
                    pexp[:], smalls[:, 0:1],
                    mybir.ActivationFunctionType.Exp, bias=0.0, scale=0.0,
                )
            # hold back the final V' chunks: they fill S(0)'s exp-paced
            # idle slots in the attention phase instead
            emit_v_transposes(min((r0 + nr) // 2, 24))

        # ---- phase C: attention, S(i) interleaved with A@V(i-1) ----
        def emit_av_epilogue(i, it, psa):
            zrec = tiny.tile([128, 1], F32, tag="zrec")
            nc.vector.reciprocal(zrec[:], psa[:, 256:257])
            rn = rn_pool.tile([128, C], F32)
            nc.vector.tensor_scalar_mul(rn[:], psa[:, 0:256], zrec[:])
            col = i * IBLK + it * 128
            scol = 4 * i + it
            for d in range(2):
                pstr = ps_tr.tile([128, 128], F32, tag="tr")
                nc.tensor.transpose(
                    pstr[:], rn[:, d * 128 : (d + 1) * 128], ident[:]
                )
                nc.vector.scalar_tensor_tensor(
                    out=y[d][:, col : col + 128],
                    in0=pstr[:],
                    scalar=1.0,
                    in1=xres[d][:, col : col + 128],
                    op0=mybir.AluOpType.mult,
                    op1=mybir.AluOpType.add,
                    accum_out=ssum[d][:, scol : scol + 1],
                )
                sq_t = sq_pool.tile([128, 128], F32)
                nc.vector.scalar_tensor_tensor(
                    out=sq_t[:],
                    in0=y[d][:, col : col + 128],
                    scalar=1.0,
                    in1=y[d][:, col : col + 128],
                    op0=mybir.AluOpType.mult,
                    op1=mybir.AluOpType.mult,
                    accum_out=ssq[d][:, scol : scol + 1],
                )

        NT2 = N_JT // 2  # 16 j-pair tiles per A@V accumulation chain

        def emit_av_steps(i, lo, hi):
            for k in range(lo, hi):
                it, t = divmod(k, NT2)
                if t == 0:
                    av_psa[i] = ps_av.tile(
                        [128, 257], F32, tag="av", name=f"psa_{i}_{it}"
                    )
                nc.tensor.matmul(
                    av_psa[i][:],
                    lhsT=et_tiles[i][:, 2 * t : 2 * t + 2,
                                     it * 128 : (it + 1) * 128],
                    rhs=vp[:, t, :, 0:257],
                    start=(t == 0),
                    stop=(t == NT2 - 1),
                    perf_mode=DR,
                )
                if t == NT2 - 1:
                    emit_av_epilogue(i, it, av_psa[i])

        for i in range(N_IBLK + 1):
            if i < N_IBLK:
                et_tiles[i] = et_pool.tile(
                    [128, N_JT, IBLK], FP8, tag="et", name=f"et_{i}"
                )
            for jp in range(16):
                if i < N_IBLK:
                    emit_s_group(i, jp)
                if i == 0 and jp >= 7:
                    emit_v_transposes(min(N_JT, vtr_done + 1))
                if i > 0:
                    emit_av_steps(i - 1, jp * 4, jp * 4 + 4)

        # ---- phase D: BN stats all-reduce + normalize + writeout ----
        prime = tiny.tile([128, 1], F32, tag="prime")
        nc.scalar.activation(
            prime[:], smalls[:, 0:1], mybir.ActivationFunctionType.Identity,
            bias=0.0, scale=1.0,
        )
        eps_t = tiny.tile([128, 1], F32, tag="eps")
        nc.vector.memset(eps_t[:], BN_EPS)
        prime2 = tiny.tile([128, 1], F32, tag="prime2")
        nc.scalar.activation(
            prime2[:], smalls[:, 6:7], mybir.ActivationFunctionType.Sqrt,
            bias=eps_t[:], scale=1.0,
        )
        partial = const.tile([128, 4], F32)  # [sum0, sum1, sq0, sq1]
        for d in range(2):
            nc.vector.tensor_reduce(
                partial[:, d : d + 1],
                ssum[d][:],
                axis=mybir.AxisListType.X,
                op=mybir.AluOpType.add,
            )
            nc.vector.tensor_reduce(
                partial[:, 2 + d : 3 + d],
                ssq[d][:],
                axis=mybir.AxisListType.X,
                op=mybir.AluOpType.add,
            )
        inb = dram.tile([128, 4], F32)
        outb = dram.tile([128, 4], F32)
        nc.sync.dma_start(out=inb[:], in_=partial[:])
        nc.gpsimd.collective_compute(
            "AllReduce",
            mybir.AluOpType.add,
            replica_groups=replica_groups,
            ins=[inb.opt()],
            outs=[outb.opt()],
        )
        g = const.tile([128, 4], F32)
        nc.sync.dma_start(out=g[:], in_=outb[:])

        inv_n = 1.0 / float(len(replica_groups[0]) * N)
        ab = const.tile([128, 4], F32)  # [a0, a1, b0, b1]
        mean2 = tiny.tile([128, 2], F32, tag="mean2")
        msq2 = tiny.tile([128, 2], F32, tag="msq2")
        nc.vector.tensor_scalar_mul(mean2[:], g[:, 0:2], inv_n)
        nc.vector.tensor_scalar_mul(msq2[:], g[:, 2:4], inv_n)
        var2 = tiny.tile([128, 2], F32, tag="var2")
        nc.vector.tensor_mul(var2[:], mean2[:], mean2[:])
        nc.vector.tensor_sub(var2[:], msq2[:], var2[:])
        std2 = tiny.tile([128, 2], F32, tag="std2")
        nc.scalar.activation(
            std2[:], var2[:], mybir.ActivationFunctionType.Sqrt,
            bias=eps_t[:], scale=1.0,
        )
        rstd2 = tiny.tile([128, 2], F32, tag="rstd2")
        nc.vector.reciprocal(rstd2[:], std2[:])
        nc.vector.tensor_mul(ab[:, 0:2], rstd2[:], smalls[:, 6:8])
        t2 = tiny.tile([128, 2], F32, tag="t2")
        nc.vector.tensor_mul(t2[:], mean2[:], ab[:, 0:2])
        nc.vector.tensor_sub(ab[:, 2:4], smalls[:, 8:10], t2[:])

        CH = 512
        for k in range(N // CH):
            sl = slice(k * CH, (k + 1) * CH)
            for d in range(2):
                a_v = ab[:, d : d + 1]
                b_v = ab[:, 2 + d : 3 + d]
                cs = slice(d * 128, (d + 1) * 128)
                o_t = sq_pool.tile([128, CH], BF16, tag="stg", bufs=8)
                if d == 0:
                    nc.scalar.activation(
                        o_t[:],
                        y[d][:, sl],
                        mybir.ActivationFunctionType.Identity,
                        bias=b_v,
                        scale=a_v,
                    )
                else:
                    nc.vector.tensor_scalar(
                        out=o_t[:],
                        in0=y[d][:, sl],
                        scalar1=a_v,
                        scalar2=b_v,
                        op0=mybir.AluOpType.mult,
                        op1=mybir.AluOpType.add,
                    )
                # spread the writeout across three DMA paths (both HWDGE
                # queue groups + gpsimd SWDGE) so the chunks drain in
                # parallel
                eng = (nc.scalar, nc.sync, nc.gpsimd)[(2 * k + d) % 3]
                eng.dma_start(out=out_d[cs, sl], in_=o_t[:])


def pack_inputs(x, wq, bq, wkv, bkv, gamma, beta):
    """Host-side packing: per-core input maps (fp8 DoubleRow layouts)."""
    import ml_dtypes

    f8 = ml_dtypes.float8_e4m3
    B = x.shape[0]
    xc = np.ascontiguousarray(x.reshape(B, C, HW, HW).astype(np.float32))
    # xpad [B, 128, 2, FLATP] fp8: channel c = p + 128*s, flat padded 66x66
    xp = np.zeros((B, 2, 128, PW, PW), np.float32)
    xp[:, 0, :, 1:-1, 1:-1] = xc[:, 0:128]
    xp[:, 1, :, 1:-1, 1:-1] = xc[:, 128:256]
    xpad = np.zeros((B, 128, 2, FLATP), f8)
    xpad[:, :, :, 0:FLAT] = np.clip(
        xp.transpose(0, 2, 1, 3, 4).reshape(B, 128, 2, FLAT), -240, 240
    ).astype(f8)
    # wkv [128, 4, 9, 2, 128]: [p, o//128, kh*3+kw, s, o%128] =
    # wkv[o, p+128s, kh, kw]; the o-group as dim1 keeps each group's DMA
    # contiguous per partition so group 0 can land first
    wkvf = np.clip(wkv.astype(np.float32), -240, 240)
    wkv_dr = np.ascontiguousarray(
        wkvf.reshape(4, 128, 2, 128, 3, 3).transpose(3, 0, 4, 5, 2, 1).reshape(
            128, 4, 9, 2, 128
        )
    ).astype(f8)
    # wq [128, 2, 256]: [p, s, o] = wq[o, p+128s]
    wqf = np.clip(wq.reshape(C, C).astype(np.float32), -240, 240)
    wq_dr = np.ascontiguousarray(
        wqf.reshape(C, 2, 128).transpose(2, 1, 0)
    ).astype(f8)
    xresb = np.ascontiguousarray(xc.reshape(B, C, N)).astype(ml_dtypes.bfloat16)
    smalls = np.zeros((128, 10), np.float32)
    smalls[:, 0] = bq[0:128]
    smalls[:, 1] = bq[128:256]
    for k in range(4):
        smalls[:, 2 + k] = bkv[k * 128 : (k + 1) * 128]
    smalls[:, 6] = gamma[0:128]
    smalls[:, 7] = gamma[128:256]
    smalls[:, 8] = beta[0:128]
    smalls[:, 9] = beta[128:256]
    return [
        {
            "xpad": xpad[b],
            "xres": xresb[b],
            "wkv": wkv_dr,
            "wq": wq_dr,
            "smalls": smalls,
        }
        for b in range(B)
    ]


_CACHED = {}


def get_program():
    if "nc" not in _CACHED:
        _CACHED["nc"] = build_program()
    return _CACHED["nc"]


def kernel(x, wq, bq, wkv, bkv, gamma, beta, trace=False):
    x = np.asarray(x)
    in_maps = pack_inputs(
        x,
        np.asarray(wq),
        np.asarray(bq),
        np.asarray(wkv),
        np.asarray(bkv),
        np.asarray(gamma),
        np.asarray(beta),
    )
    nc = get_program()
    try:
        res = run_bass_kernel_spmd(
            nc, in_maps, core_ids=list(range(N_CORES)), trace=trace
        )
    except Exception:
        # a wedged axon terminal (LoadExecutable/exec errors) is recoverable
        import ctypes

        try:
            lib = ctypes.CDLL("/opt/axon/libaxon_pjrt.so")
            lib.axon_reset.restype = ctypes.c_int64
            lib.axon_reset()
        except Exception:
            pass
        res = run_bass_kernel_spmd(
            nc, in_maps, core_ids=list(range(N_CORES)), trace=trace
        )
    out = np.stack(
        [
            res.results[b]["out"].astype(np.float32).reshape(C, HW, HW)
            for b in range(N_CORES)
        ]
    )
    if trace:
        kernel.last_results = res
    return out


# revision 41
# speedup vs baseline: 1.2018x; 1.2018x over previous
"""Bass/Trainium2 kernel for nn_ExpressionEncoder (conv-QKV attention + BN).

Data-parallel over batch: 8 images -> 8 NeuronCores, one image per core.

v2: fp8 (TRN e4m3) DoubleRow matmuls for the convs and the S^T (logits)
matmul -- K=256 contraction per instruction at ~1.8x the bf16 rate (the
PE runs ~2.0 GHz under full-chip load while LDWEIGHTS stays on the
1.2 GHz NX clock, so DoubleRow's 2-wide rows win big). Numerics
validated offline vs the fp32 reference: l2 ~4e-3 (tolerance 2e-2).

Per-core pipeline (everything on-chip between input DMA and output DMA):
  1. Host packs x twice: fp8 xpad [128, 2, 4368] (channel-pair conv
     input, flat padded 66x66 rows) and bf16 x [256, 4096] (residual).
     Weights are packed as DoubleRow pairs: wkv [128, 9, 2, 512],
     wq [128, 2, 256].
  2. KV conv: per 128-channel output group and 7-row block, 9
     accumulating DoubleRow matmuls over contiguous flat windows
     (seam columns between rows compute garbage and are skipped by the
     relu's strided read). K -> kt fp8 [128, 2, 4096]; V -> fp32 vt,
     PE-transposed into V' [j, 257] bf16 with a trailing ones column
     (softmax denominator falls out of the A@V matmul for free).
     Q (1x1 conv) -> qt fp8 [128, 2, 4096] the same way.
  3. Attention, software-pipelined per 512-query block i: the 32
     S^T DoubleRow matmuls of block i are interleaved with the 128
     bf16 A@V matmuls of block i-1, so the PE stays busy while ScalarE
     exps block i (exp output is fp8 with exp(x/16 - 9) -- scale
     cancels in the softmax ratio; max logit ~12.8 so no overflow at
     the TRN e4m3 +-240 clip). A@V + ones column -> normalize ->
     PE-transpose back to [d, i] -> residual add -> y fp32; BN partial
     sums ride the same DVE op via accum_out.
  4. AllReduce (8 cores) of per-channel [sum(y), sum(y^2)] -> scale a,
     bias b -> out = a*y + b in bf16 (chunks alternate ScalarE/VectorE,
     chunk DMAs split across both HWDGE queue groups; host upcasts to
     fp32). A warmup AllReduce runs during the conv so the real one
     doesn't pay cold ALGO_MESH setup.

Scheduling notes (measured on HW): keep the PE transposes exactly at
their natural drain points -- deferring them to fill visible PE gaps
breaks the LDWEIGHTS background-buffer pipelining and inflates every
matmul in the stream by ~10%. The attention phase is bounded by
LDWEIGHTS column throughput (NX at 1.2 GHz): A@V reloads a 256-col
stationary per 257 moving columns, which is unavoidable while the
softmax denominator rides the V' ones column (operand-flipping A@V
would need a cross-partition reduction for Z instead).
"""

import os
import sys

for _p in ("/opt/trn_rl_repo", os.path.expanduser("~/.axon_site/_ro/trn_rl_repo")):
    if os.path.isdir(_p) and _p not in sys.path:
        sys.path.append(_p)

import numpy as np

import concourse.bass as bass
import concourse.tile as tile
from concourse import bacc, mybir
from concourse.bass_utils import run_bass_kernel_spmd
from concourse.masks import make_identity

dt = mybir.dt
F32 = dt.float32
BF16 = dt.bfloat16
FP8 = dt.float8e4

N_CORES = 8
C = 256        # channels (= dm)
HW = 64        # spatial side
N = HW * HW    # tokens per image
PW = HW + 2    # padded side
FLAT = PW * PW          # 4356
FLATP = 4368            # padded to a 16-multiple for DoubleRow strides
IBLK = 512
N_IBLK = N // IBLK      # 8
N_JT = N // 128         # 32
BN_EPS = 1e-5
INV_SQRT_DM = 1.0 / 16.0
EXP_BIAS = -9.0         # exp(sim/16 - 9): keeps fp8 et under the 240 clip
# conv row blocks: (first output row, rows). 7-row blocks have a 460-wide
# flat output window; the final 8 rows go in two 4-row blocks (262-wide)
# to keep the matmul free dim >= 256.
BLOCKS = [(0, 7), (7, 7), (14, 7), (21, 7), (28, 7), (35, 7), (42, 7),
          (49, 7), (56, 4), (60, 4)]
DR = mybir.MatmulPerfMode.DoubleRow


def build_program(n_cores=N_CORES, replica_groups=None):
    if replica_groups is None:
        replica_groups = [list(range(n_cores))]
    nc = bacc.Bacc(
        "TRN2", target_bir_lowering=False, debug=False, num_devices=n_cores
    )
    xpad_d = nc.dram_tensor("xpad", [128, 2, FLATP], FP8, kind="ExternalInput")
    xres_d = nc.dram_tensor("xres", [C, N], BF16, kind="ExternalInput")
    wkv_d = nc.dram_tensor("wkv", [128, 4, 9, 2, 128], FP8, kind="ExternalInput")
    wq_d = nc.dram_tensor("wq", [128, 2, C], FP8, kind="ExternalInput")
    smalls_d = nc.dram_tensor("smalls", [128, 10], F32, kind="ExternalInput")
    out_d = nc.dram_tensor("out", [C, N], BF16, kind="ExternalOutput")

    with tile.TileContext(nc) as tc:
        _body(tc, xpad_d, xres_d, wkv_d, wq_d, smalls_d, out_d, replica_groups)
    nc.compile()
    return nc


def _body(tc, xpad_d, xres_d, wkv_d, wq_d, smalls_d, out_d, replica_groups):
    nc = tc.nc
    from contextlib import ExitStack

    ctx = ExitStack()
    with ctx:
        const = ctx.enter_context(tc.tile_pool(name="const", bufs=1))
        et_pool = ctx.enter_context(tc.tile_pool(name="et", bufs=2))
        rn_pool = ctx.enter_context(tc.tile_pool(name="rn", bufs=2))
        sq_pool = ctx.enter_context(tc.tile_pool(name="sq", bufs=2))
        tiny = ctx.enter_context(tc.tile_pool(name="tiny", bufs=2))
        dram = ctx.enter_context(tc.tile_pool(name="dram", bufs=1, space="DRAM"))
        ps_mm = ctx.enter_context(tc.tile_pool(name="ps_mm", bufs=2, space="PSUM"))
        ps_av = ctx.enter_context(tc.tile_pool(name="ps_av", bufs=2, space="PSUM"))
        ps_tr = ctx.enter_context(tc.tile_pool(name="ps_tr", bufs=2, space="PSUM"))

        # HAM pre-warm: the PE clock-gate needs ~3.4us of sustained matmul
        # activity to go 4/8 -> 8/8; burn dummy matmuls on a zeroed tile
        # during the input-DMA gate so the first real conv matmuls run at
        # full clock
        warm_w = const.tile([128, 128], BF16)
        nc.vector.memset(warm_w[:], 0.0)
        ps_warm = ps_mm.tile([128, 1024], F32, tag="mm", name="ps_warm")
        for k in range(72):
            nc.tensor.matmul(
                ps_warm[:, 0:128],
                lhsT=warm_w[:],
                rhs=warm_w[:],
                start=(k == 0),
                stop=(k == 71),
            )

        # ---- inputs (conv inputs first -- they gate the PE start) ----
        xpad = const.tile([128, 2, FLATP], FP8)
        wkv_sb = const.tile([128, 4, 9, 2, 128], FP8)
        for s in range(2):
            nc.sync.dma_start(out=xpad[:, s, 0:594], in_=xpad_d[:, s, 0:594])
        for g in range(4):
            nc.sync.dma_start(out=wkv_sb[:, g, :, :, :], in_=wkv_d[:, g, :, :, :])
        for s in range(2):
            nc.sync.dma_start(out=xpad[:, s, 594:2184], in_=xpad_d[:, s, 594:2184])
        for s in range(2):
            nc.sync.dma_start(out=xpad[:, s, 2184:FLATP], in_=xpad_d[:, s, 2184:FLATP])
        smalls = const.tile([128, 10], F32)
        nc.sync.dma_start(out=smalls[:], in_=smalls_d[:])
        ident = const.tile([128, 128], F32)
        make_identity(nc, ident[:])
        wq_sb = const.tile([128, 2, C], FP8)
        nc.sync.dma_start(out=wq_sb[:], in_=wq_d[:])
        xres = [const.tile([128, N], BF16, name=f"xres{ct}", tag=f"xres{ct}")
                for ct in range(2)]
        for ct in range(2):
            cs = slice(ct * 128, (ct + 1) * 128)
            for hc in range(2):
                nc.sync.dma_start(
                    out=xres[ct][:, hc * 2048 : (hc + 1) * 2048],
                    in_=xres_d[cs, hc * 2048 : (hc + 1) * 2048],
                )

        # warm up the collectives firmware during the conv so the real BN
        # all-reduce doesn't pay the ~11us cold ALGO_MESH setup
        wu_sb = tiny.tile([128, 1], F32, tag="wu")
        nc.vector.memset(wu_sb[:], 0.0)
        wu_in = dram.tile([128, 1], F32)
        wu_out = dram.tile([128, 1], F32)
        nc.sync.dma_start(out=wu_in[:], in_=wu_sb[:])
        nc.gpsimd.collective_compute(
            "AllReduce",
            mybir.AluOpType.add,
            replica_groups=replica_groups,
            ins=[wu_in.opt()],
            outs=[wu_out.opt()],
        )
        wu_back = tiny.tile([128, 1], F32, tag="wub")
        nc.sync.dma_start(out=wu_back[:], in_=wu_out[:])

        # ---- persistent activations ----
        kt = const.tile([128, 2, N], FP8)
        qt = const.tile([128, 2, N], FP8)
        vt_dr = [const.tile([128, N], F32, name=f"vt{d}", tag=f"vt{d}")
                 for d in range(2)]
        # V' [j, d + ones] in fp8 DoubleRow pairs: [j_lo, t, j_hi, d] with
        # j = j_lo + 128 * (2t + j_hi); stride 272 keeps the pair step
        # 16-aligned
        vp = const.tile([128, N_JT // 2, 2, 272], FP8)
        nc.vector.memset(vp[:, :, :, 256], 1.0)
        y = [const.tile([128, N], F32, name=f"y{d}", tag=f"y{d}")
             for d in range(2)]
        ebias = const.tile([128, 1], F32)
        nc.vector.memset(ebias[:], EXP_BIAS)
        ssum = [const.tile([128, 4 * N_IBLK], F32, name=f"ssum{d}", tag=f"ssum{d}")
                for d in range(2)]
        ssq = [const.tile([128, 4 * N_IBLK], F32, name=f"ssq{d}", tag=f"ssq{d}")
               for d in range(2)]

        # ---- attention helpers (S^T block 0 overlaps the conv phase) ----
        et_tiles = {}
        av_psa = {}

        def emit_s_group(i, jp):
            pst = ps_mm.tile([128, 1024], F32, tag="mm")
            i0 = i * IBLK
            for sub in range(2):
                jt = 2 * jp + sub
                nc.tensor.matmul(
                    pst[:, sub * 512 : (sub + 1) * 512],
                    lhsT=kt[:, :, jt * 128 : (jt + 1) * 128],
                    rhs=qt[:, :, i0 : i0 + IBLK],
                    start=True,
                    stop=True,
                    perf_mode=DR,
                )
            nc.scalar.activation(
                et_tiles[i][:, 2 * jp : 2 * jp + 2, :],
                pst[:].rearrange("p (a b) -> p a b", a=2),
                mybir.ActivationFunctionType.Exp,
                bias=ebias[:],
                scale=INV_SQRT_DM,
            )

        # ---- phase B: Q/KV convs (+ V transposes as 128-j chunks land) ----
        shifts = [(kh, kw) for kh in range(3) for kw in range(3)]
        vtr_done = 0

        def emit_v_transposes(upto):
            nonlocal vtr_done
            for cch in range(vtr_done, upto):
                for dv in range(2):
                    pstr = ps_tr.tile([128, 128], F32, tag="tr")
                    nc.tensor.transpose(
                        pstr[:], vt_dr[dv][:, cch * 128 : (cch + 1) * 128], ident[:]
                    )
                    nc.vector.tensor_copy(
                        vp[:, cch // 2, cch % 2, dv * 128 : (dv + 1) * 128],
                        pstr[:],
                    )
            vtr_done = upto

        for bi, (r0, nr) in enumerate(BLOCKS):
            L = (nr - 1) * 66 + 64
            for dkvt in range(4):
                ps = ps_mm.tile([128, 1024], F32, tag="mm")
                for si, (sh, sw) in enumerate(shifts):
                    base = (r0 + sh) * 66 + sw
                    nc.tensor.matmul(
                        ps[:, 0:L],
                        lhsT=wkv_sb[:, dkvt, si, :, :],
                        rhs=xpad[:, :, base : base + L],
                        start=(si == 0),
                        stop=(si == 8),
                        perf_mode=DR,
                    )
                src = ps[:, 0 : nr * 66].rearrange("p (r w) -> p r w", w=66)[
                    :, :, 0:64
                ]
                bias = smalls[:, 2 + dkvt : 3 + dkvt]
                if dkvt < 2:
                    dst = kt[:, dkvt, r0 * 64 : (r0 + nr) * 64].rearrange(
                        "p (r w) -> p r w", w=64
                    )
                else:
                    dst = vt_dr[dkvt - 2][:, r0 * 64 : (r0 + nr) * 64].rearrange(
                        "p (r w) -> p r w", w=64
                    )
                nc.scalar.activation(
                    dst, src, mybir.ActivationFunctionType.Relu,
                    bias=bias, scale=1.0,
                )
            for dqt in range(2):
                psq = ps_mm.tile([128, 1024], F32, tag="mm")
                base = (r0 + 1) * 66 + 1
                nc.tensor.matmul(
                    psq[:, 0:L],
                    lhsT=wq_sb[:, :, dqt * 128 : (dqt + 1) * 128],
                    rhs=xpad[:, :, base : base + L],
                    start=True,
                    stop=True,
                    perf_mode=DR,
                )
                srcq = psq[:, 0 : nr * 66].rearrange("p (r w) -> p r w", w=66)[
                    :, :, 0:64
                ]
                dstq = qt[:, dqt, r0 * 64 : (r0 + nr) * 64].rearrange(
                    "p (r w) -> p r w", w=64
                )
                nc.scalar.activation(
                    dstq, srcq, mybir.ActivationFunctionType.Relu,
                    bias=smalls[:, dqt : dqt + 1], scale=1.0,
                )
            if bi == 0:
                # prime the Exp ACT table while the PE is busy with conv
                pexp = tiny.tile([128, 1], F32, tag="pexp")
                nc.scalar.activation(
                    pexp[:], smalls[:, 0:1],
                    mybir.ActivationFunctionType.Exp, bias=0.0, scale=0.0,
                )
            emit_v_transposes((r0 + nr) // 2)

        # ---- phase C: attention, S(i) interleaved with A@V(i-1) ----
        def emit_av_epilogue(i, it, psa):
            zrec = tiny.tile([128, 1], F32, tag="zrec")
            nc.vector.reciprocal(zrec[:], psa[:, 256:257])
            rn = rn_pool.tile([128, C], F32)
            nc.vector.tensor_scalar_mul(rn[:], psa[:, 0:256], zrec[:])
            col = i * IBLK + it * 128
            scol = 4 * i + it
            for d in range(2):
                pstr = ps_tr.tile([128, 128], F32, tag="tr")
                nc.tensor.transpose(
                    pstr[:], rn[:, d * 128 : (d + 1) * 128], ident[:]
                )
                nc.vector.scalar_tensor_tensor(
                    out=y[d][:, col : col + 128],
                    in0=pstr[:],
                    scalar=1.0,
                    in1=xres[d][:, col : col + 128],
                    op0=mybir.AluOpType.mult,
                    op1=mybir.AluOpType.add,
                    accum_out=ssum[d][:, scol : scol + 1],
                )
                sq_t = sq_pool.tile([128, 128], F32)
                nc.vector.scalar_tensor_tensor(
                    out=sq_t[:],
                    in0=y[d][:, col : col + 128],
                    scalar=1.0,
                    in1=y[d][:, col : col + 128],
                    op0=mybir.AluOpType.mult,
                    op1=mybir.AluOpType.mult,
                    accum_out=ssq[d][:, scol : scol + 1],
                )

        NT2 = N_JT // 2  # 16 j-pair tiles per A@V accumulation chain

        def emit_av_steps(i, lo, hi):
            for k in range(lo, hi):
                it, t = divmod(k, NT2)
                if t == 0:
                    av_psa[i] = ps_av.tile(
                        [128, 257], F32, tag="av", name=f"psa_{i}_{it}"
                    )
                nc.tensor.matmul(
                    av_psa[i][:],
                    lhsT=et_tiles[i][:, 2 * t : 2 * t + 2,
                                     it * 128 : (it + 1) * 128],
                    rhs=vp[:, t, :, 0:257],
                    start=(t == 0),
                    stop=(t == NT2 - 1),
                    perf_mode=DR,
                )
                if t == NT2 - 1:
                    emit_av_epilogue(i, it, av_psa[i])

        for i in range(N_IBLK + 1):
            if i < N_IBLK:
                et_tiles[i] = et_pool.tile(
                    [128, N_JT, IBLK], FP8, tag="et", name=f"et_{i}"
                )
            for jp in range(16):
                if i < N_IBLK:
                    emit_s_group(i, jp)
                if i > 0:
                    emit_av_steps(i - 1, jp * 4, jp * 4 + 4)

        # ---- phase D: BN stats all-reduce + normalize + writeout ----
        prime = tiny.tile([128, 1], F32, tag="prime")
        nc.scalar.activation(
            prime[:], smalls[:, 0:1], mybir.ActivationFunctionType.Identity,
            bias=0.0, scale=1.0,
        )
        eps_t = tiny.tile([128, 1], F32, tag="eps")
        nc.vector.memset(eps_t[:], BN_EPS)
        prime2 = tiny.tile([128, 1], F32, tag="prime2")
        nc.scalar.activation(
            prime2[:], smalls[:, 6:7], mybir.ActivationFunctionType.Sqrt,
            bias=eps_t[:], scale=1.0,
        )
        partial = const.tile([128, 4], F32)  # [sum0, sum1, sq0, sq1]
        for d in range(2):
            nc.vector.tensor_reduce(
                partial[:, d : d + 1],
                ssum[d][:],
                axis=mybir.AxisListType.X,
                op=mybir.AluOpType.add,
            )
            nc.vector.tensor_reduce(
                partial[:, 2 + d : 3 + d],
                ssq[d][:],
                axis=mybir.AxisListType.X,
                op=mybir.AluOpType.add,
            )
        inb = dram.tile([128, 4], F32)
        outb = dram.tile([128, 4], F32)
        nc.sync.dma_start(out=inb[:], in_=partial[:])
        nc.gpsimd.collective_compute(
            "AllReduce",
            mybir.AluOpType.add,
            replica_groups=replica_groups,
            ins=[inb.opt()],
            outs=[outb.opt()],
        )
        g = const.tile([128, 4], F32)
        nc.sync.dma_start(out=g[:], in_=outb[:])

        inv_n = 1.0 / float(len(replica_groups[0]) * N)
        ab = const.tile([128, 4], F32)  # [a0, a1, b0, b1]
        mean2 = tiny.tile([128, 2], F32, tag="mean2")
        msq2 = tiny.tile([128, 2], F32, tag="msq2")
        nc.vector.tensor_scalar_mul(mean2[:], g[:, 0:2], inv_n)
        nc.vector.tensor_scalar_mul(msq2[:], g[:, 2:4], inv_n)
        var2 = tiny.tile([128, 2], F32, tag="var2")
        nc.vector.tensor_mul(var2[:], mean2[:], mean2[:])
        nc.vector.tensor_sub(var2[:], msq2[:], var2[:])
        std2 = tiny.tile([128, 2], F32, tag="std2")
        nc.scalar.activation(
            std2[:], var2[:], mybir.ActivationFunctionType.Sqrt,
            bias=eps_t[:], scale=1.0,
        )
        rstd2 = tiny.tile([128, 2], F32, tag="rstd2")
        nc.vector.reciprocal(rstd2[:], std2[:])
        nc.vector.tensor_mul(ab[:, 0:2], rstd2[:], smalls[:, 6:8])
        t2 = tiny.tile([128, 2], F32, tag="t2")
        nc.vector.tensor_mul(t2[:], mean2[:], ab[:, 0:2])
        nc.vector.tensor_sub(ab[:, 2:4], smalls[:, 8:10], t2[:])

        CH = 512
        for k in range(N // CH):
            sl = slice(k * CH, (k + 1) * CH)
            for d in range(2):
                a_v = ab[:, d : d + 1]
                b_v = ab[:, 2 + d : 3 + d]
                cs = slice(d * 128, (d + 1) * 128)
                o_t = sq_pool.tile([128, CH], BF16, tag="stg", bufs=8)
                if d == 0:
                    nc.scalar.activation(
                        o_t[:],
                        y[d][:, sl],
                        mybir.ActivationFunctionType.Identity,
                        bias=b_v,
                        scale=a_v,
                    )
                else:
                    nc.vector.tensor_scalar(
                        out=o_t[:],
                        in0=y[d][:, sl],
                        scalar1=a_v,
                        scalar2=b_v,
                        op0=mybir.AluOpType.mult,
                        op1=mybir.AluOpType.add,
                    )
                # spread the writeout across three DMA paths (both HWDGE
                # queue groups + gpsimd SWDGE) so the chunks drain in
                # parallel
                eng = (nc.scalar, nc.sync, nc.gpsimd)[(2 * k + d) % 3]
                eng.dma_start(out=out_d[cs, sl], in_=o_t[:])


def pack_inputs(x, wq, bq, wkv, bkv, gamma, beta):
    """Host-side packing: per-core input maps (fp8 DoubleRow layouts)."""
    import ml_dtypes

    f8 = ml_dtypes.float8_e4m3
    B = x.shape[0]
    xc = np.ascontiguousarray(x.reshape(B, C, HW, HW).astype(np.float32))
    # xpad [B, 128, 2, FLATP] fp8: channel c = p + 128*s, flat padded 66x66
    xp = np.zeros((B, 2, 128, PW, PW), np.float32)
    xp[:, 0, :, 1:-1, 1:-1] = xc[:, 0:128]
    xp[:, 1, :, 1:-1, 1:-1] = xc[:, 128:256]
    xpad = np.zeros((B, 128, 2, FLATP), f8)
    xpad[:, :, :, 0:FLAT] = np.clip(
        xp.transpose(0, 2, 1, 3, 4).reshape(B, 128, 2, FLAT), -240, 240
    ).astype(f8)
    # wkv [128, 4, 9, 2, 128]: [p, o//128, kh*3+kw, s, o%128] =
    # wkv[o, p+128s, kh, kw]; the o-group as dim1 keeps each group's DMA
    # contiguous per partition so group 0 can land first
    wkvf = np.clip(wkv.astype(np.float32), -240, 240)
    wkv_dr = np.ascontiguousarray(
        wkvf.reshape(4, 128, 2, 128, 3, 3).transpose(3, 0, 4, 5, 2, 1).reshape(
            128, 4, 9, 2, 128
        )
    ).astype(f8)
    # wq [128, 2, 256]: [p, s, o] = wq[o, p+128s]
    wqf = np.clip(wq.reshape(C, C).astype(np.float32), -240, 240)
    wq_dr = np.ascontiguousarray(
        wqf.reshape(C, 2, 128).transpose(2, 1, 0)
    ).astype(f8)
    xresb = np.ascontiguousarray(xc.reshape(B, C, N)).astype(ml_dtypes.bfloat16)
    smalls = np.zeros((128, 10), np.float32)
    smalls[:, 0] = bq[0:128]
    smalls[:, 1] = bq[128:256]
    for k in range(4):
        smalls[:, 2 + k] = bkv[k * 128 : (k + 1) * 128]
    smalls[:, 6] = gamma[0:128]
    smalls[:, 7] = gamma[128:256]
    smalls[:, 8] = beta[0:128]
    smalls[:, 9] = beta[128:256]
    return [
        {
            "xpad": xpad[b],
            "xres": xresb[b],
            "wkv": wkv_dr,
            "wq": wq_dr,
            "smalls": smalls,
        }
        for b in range(B)
    ]


_CACHED = {}


def get_program():
    if "nc" not in _CACHED:
        _CACHED["nc"] = build_program()
    return _CACHED["nc"]


def kernel(x, wq, bq, wkv, bkv, gamma, beta, trace=False):
    x = np.asarray(x)
    in_maps = pack_inputs(
        x,
        np.asarray(wq),
        np.asarray(bq),
        np.asarray(wkv),
        np.asarray(bkv),
        np.asarray(gamma),
        np.asarray(beta),
    )
    nc = get_program()
    try:
        res = run_bass_kernel_spmd(
            nc, in_maps, core_ids=list(range(N_CORES)), trace=trace
        )
    except Exception:
        # a wedged axon terminal (LoadExecutable/exec errors) is recoverable
        import ctypes

        try:
            lib = ctypes.CDLL("/opt/axon/libaxon_pjrt.so")
            lib.axon_reset.restype = ctypes.c_int64
            lib.axon_reset()
        except Exception:
            pass
        res = run_bass_kernel_spmd(
            nc, in_maps, core_ids=list(range(N_CORES)), trace=trace
        )
    out = np.stack(
        [
            res.results[b]["out"].astype(np.float32).reshape(C, HW, HW)
            for b in range(N_CORES)
        ]
    )
    if trace:
        kernel.last_results = res
    return out


# revision 44
# speedup vs baseline: 1.2057x; 1.0032x over previous
"""Bass/Trainium2 kernel for nn_ExpressionEncoder (conv-QKV attention + BN).

Data-parallel over batch: 8 images -> 8 NeuronCores, one image per core.

v2: fp8 (TRN e4m3) DoubleRow matmuls for the convs and the S^T (logits)
matmul -- K=256 contraction per instruction at ~1.8x the bf16 rate (the
PE runs ~2.0 GHz under full-chip load while LDWEIGHTS stays on the
1.2 GHz NX clock, so DoubleRow's 2-wide rows win big). Numerics
validated offline vs the fp32 reference: l2 ~4e-3 (tolerance 2e-2).

Per-core pipeline (everything on-chip between input DMA and output DMA):
  1. Host packs x twice: fp8 xpad [128, 2, 4368] (channel-pair conv
     input, flat padded 66x66 rows) and bf16 x [256, 4096] (residual).
     Weights are packed as DoubleRow pairs: wkv [128, 9, 2, 512],
     wq [128, 2, 256].
  2. KV conv: per 128-channel output group and 7-row block, 9
     accumulating DoubleRow matmuls over contiguous flat windows
     (seam columns between rows compute garbage and are skipped by the
     relu's strided read). K -> kt fp8 [128, 2, 4096]; V -> fp32 vt,
     PE-transposed into V' [j, 257] bf16 with a trailing ones column
     (softmax denominator falls out of the A@V matmul for free).
     Q (1x1 conv) -> qt fp8 [128, 2, 4096] the same way.
  3. Attention, software-pipelined per 512-query block i: the 32
     S^T DoubleRow matmuls of block i are interleaved with the 128
     bf16 A@V matmuls of block i-1, so the PE stays busy while ScalarE
     exps block i (exp output is fp8 with exp(x/16 - 9) -- scale
     cancels in the softmax ratio; max logit ~12.8 so no overflow at
     the TRN e4m3 +-240 clip). A@V + ones column -> normalize ->
     PE-transpose back to [d, i] -> residual add -> y fp32; BN partial
     sums ride the same DVE op via accum_out.
  4. AllReduce (8 cores) of per-channel [sum(y), sum(y^2)] -> scale a,
     bias b -> out = a*y + b in bf16 (chunks alternate ScalarE/VectorE,
     chunk DMAs split across both HWDGE queue groups; host upcasts to
     fp32). A warmup AllReduce runs during the conv so the real one
     doesn't pay cold ALGO_MESH setup.

Scheduling notes (measured on HW): keep the PE transposes exactly at
their natural drain points -- deferring them to fill visible PE gaps
breaks the LDWEIGHTS background-buffer pipelining and inflates every
matmul in the stream by ~10%. The attention phase is bounded by
LDWEIGHTS column throughput (NX at 1.2 GHz): A@V reloads a 256-col
stationary per 257 moving columns, which is unavoidable while the
softmax denominator rides the V' ones column (operand-flipping A@V
would need a cross-partition reduction for Z instead).
"""

import os
import sys

for _p in ("/opt/trn_rl_repo", os.path.expanduser("~/.axon_site/_ro/trn_rl_repo")):
    if os.path.isdir(_p) and _p not in sys.path:
        sys.path.append(_p)

import numpy as np

import concourse.bass as bass
import concourse.tile as tile
from concourse import bacc, mybir
from concourse.bass_utils import run_bass_kernel_spmd
from concourse.masks import make_identity

dt = mybir.dt
F32 = dt.float32
BF16 = dt.bfloat16
FP8 = dt.float8e4

N_CORES = 8
C = 256        # channels (= dm)
HW = 64        # spatial side
N = HW * HW    # tokens per image
PW = HW + 2    # padded side
FLAT = PW * PW          # 4356
FLATP = 4368            # padded to a 16-multiple for DoubleRow strides
IBLK = 512
N_IBLK = N // IBLK      # 8
N_JT = N // 128         # 32
BN_EPS = 1e-5
INV_SQRT_DM = 1.0 / 16.0
EXP_BIAS = -9.0         # exp(sim/16 - 9): keeps fp8 et under the 240 clip
# conv row blocks: (first output row, rows). 7-row blocks have a 460-wide
# flat output window; the final 8 rows go in two 4-row blocks (262-wide)
# to keep the matmul free dim >= 256.
BLOCKS = [(0, 7), (7, 7), (14, 7), (21, 7), (28, 7), (35, 7), (42, 7),
          (49, 7), (56, 4), (60, 4)]
DR = mybir.MatmulPerfMode.DoubleRow


def build_program(n_cores=N_CORES, replica_groups=None):
    if replica_groups is None:
        replica_groups = [list(range(n_cores))]
    nc = bacc.Bacc(
        "TRN2", target_bir_lowering=False, debug=False, num_devices=n_cores
    )
    xpad_d = nc.dram_tensor("xpad", [128, 2, FLATP], FP8, kind="ExternalInput")
    xres_d = nc.dram_tensor("xres", [C, N], BF16, kind="ExternalInput")
    wkv_d = nc.dram_tensor("wkv", [128, 4, 9, 2, 128], FP8, kind="ExternalInput")
    wq_d = nc.dram_tensor("wq", [128, 2, C], FP8, kind="ExternalInput")
    smalls_d = nc.dram_tensor("smalls", [128, 10], F32, kind="ExternalInput")
    out_d = nc.dram_tensor("out", [C, N], BF16, kind="ExternalOutput")

    with tile.TileContext(nc) as tc:
        _body(tc, xpad_d, xres_d, wkv_d, wq_d, smalls_d, out_d, replica_groups)
    nc.compile()
    return nc


def _body(tc, xpad_d, xres_d, wkv_d, wq_d, smalls_d, out_d, replica_groups):
    nc = tc.nc
    from contextlib import ExitStack

    ctx = ExitStack()
    with ctx:
        const = ctx.enter_context(tc.tile_pool(name="const", bufs=1))
        et_pool = ctx.enter_context(tc.tile_pool(name="et", bufs=2))
        rn_pool = ctx.enter_context(tc.tile_pool(name="rn", bufs=2))
        sq_pool = ctx.enter_context(tc.tile_pool(name="sq", bufs=2))
        tiny = ctx.enter_context(tc.tile_pool(name="tiny", bufs=2))
        dram = ctx.enter_context(tc.tile_pool(name="dram", bufs=1, space="DRAM"))
        ps_mm = ctx.enter_context(tc.tile_pool(name="ps_mm", bufs=2, space="PSUM"))
        ps_av = ctx.enter_context(tc.tile_pool(name="ps_av", bufs=2, space="PSUM"))
        ps_tr = ctx.enter_context(tc.tile_pool(name="ps_tr", bufs=2, space="PSUM"))

        # HAM pre-warm: the PE clock-gate needs ~3.4us of sustained matmul
        # activity to go 4/8 -> 8/8; burn dummy matmuls on a zeroed tile
        # during the input-DMA gate so the first real conv matmuls run at
        # full clock
        warm_w = const.tile([128, 128], BF16)
        nc.vector.memset(warm_w[:], 0.0)
        ps_warm = ps_mm.tile([128, 1024], F32, tag="mm", name="ps_warm")
        for k in range(72):
            nc.tensor.matmul(
                ps_warm[:, 0:128],
                lhsT=warm_w[:],
                rhs=warm_w[:],
                start=(k == 0),
                stop=(k == 71),
            )

        # ---- inputs (conv inputs first -- they gate the PE start) ----
        xpad = const.tile([128, 2, FLATP], FP8)
        wkv_sb = const.tile([128, 4, 9, 2, 128], FP8)
        for s in range(2):
            nc.sync.dma_start(out=xpad[:, s, 0:594], in_=xpad_d[:, s, 0:594])
        for g in range(4):
            nc.sync.dma_start(out=wkv_sb[:, g, :, :, :], in_=wkv_d[:, g, :, :, :])
        for s in range(2):
            nc.sync.dma_start(out=xpad[:, s, 594:2184], in_=xpad_d[:, s, 594:2184])
        for s in range(2):
            nc.sync.dma_start(out=xpad[:, s, 2184:FLATP], in_=xpad_d[:, s, 2184:FLATP])
        smalls = const.tile([128, 10], F32)
        nc.sync.dma_start(out=smalls[:], in_=smalls_d[:])
        ident = const.tile([128, 128], F32)
        make_identity(nc, ident[:])
        wq_sb = const.tile([128, 2, C], FP8)
        nc.sync.dma_start(out=wq_sb[:], in_=wq_d[:])
        xres = [const.tile([128, N], BF16, name=f"xres{ct}", tag=f"xres{ct}")
                for ct in range(2)]
        for ct in range(2):
            cs = slice(ct * 128, (ct + 1) * 128)
            for hc in range(2):
                nc.sync.dma_start(
                    out=xres[ct][:, hc * 2048 : (hc + 1) * 2048],
                    in_=xres_d[cs, hc * 2048 : (hc + 1) * 2048],
                )

        # warm up the collectives firmware during the conv so the real BN
        # all-reduce doesn't pay the ~11us cold ALGO_MESH setup
        wu_sb = tiny.tile([128, 1], F32, tag="wu")
        nc.vector.memset(wu_sb[:], 0.0)
        wu_in = dram.tile([128, 1], F32)
        wu_out = dram.tile([128, 1], F32)
        nc.sync.dma_start(out=wu_in[:], in_=wu_sb[:])
        nc.gpsimd.collective_compute(
            "AllReduce",
            mybir.AluOpType.add,
            replica_groups=replica_groups,
            ins=[wu_in.opt()],
            outs=[wu_out.opt()],
        )
        wu_back = tiny.tile([128, 1], F32, tag="wub")
        nc.sync.dma_start(out=wu_back[:], in_=wu_out[:])

        # ---- persistent activations ----
        kt = const.tile([128, 2, N], FP8)
        qt = const.tile([128, 2, N], FP8)
        vt_dr = [const.tile([128, N], F32, name=f"vt{d}", tag=f"vt{d}")
                 for d in range(2)]
        # V' [j, d + ones] in fp8 DoubleRow pairs: [j_lo, t, j_hi, d] with
        # j = j_lo + 128 * (2t + j_hi); stride 272 keeps the pair step
        # 16-aligned
        vp = const.tile([128, N_JT // 2, 2, 272], FP8)
        nc.vector.memset(vp[:, :, :, 256], 1.0)
        y = [const.tile([128, N], F32, name=f"y{d}", tag=f"y{d}")
             for d in range(2)]
        ebias = const.tile([128, 1], F32)
        nc.vector.memset(ebias[:], EXP_BIAS)
        ssum = [const.tile([128, 4 * N_IBLK], F32, name=f"ssum{d}", tag=f"ssum{d}")
                for d in range(2)]
        ssq = [const.tile([128, 4 * N_IBLK], F32, name=f"ssq{d}", tag=f"ssq{d}")
               for d in range(2)]

        # ---- attention helpers (S^T block 0 overlaps the conv phase) ----
        et_tiles = {}
        av_psa = {}

        def emit_s_group(i, jp):
            pst = ps_mm.tile([128, 1024], F32, tag="mm")
            i0 = i * IBLK
            for sub in range(2):
                jt = 2 * jp + sub
                nc.tensor.matmul(
                    pst[:, sub * 512 : (sub + 1) * 512],
                    lhsT=kt[:, :, jt * 128 : (jt + 1) * 128],
                    rhs=qt[:, :, i0 : i0 + IBLK],
                    start=True,
                    stop=True,
                    perf_mode=DR,
                )
            nc.scalar.activation(
                et_tiles[i][:, 2 * jp : 2 * jp + 2, :],
                pst[:].rearrange("p (a b) -> p a b", a=2),
                mybir.ActivationFunctionType.Exp,
                bias=ebias[:],
                scale=INV_SQRT_DM,
            )

        # ---- phase B: Q/KV convs (+ V transposes as 128-j chunks land) ----
        shifts = [(kh, kw) for kh in range(3) for kw in range(3)]
        vtr_done = 0

        def emit_v_transposes(upto):
            nonlocal vtr_done
            for cch in range(vtr_done, upto):
                for dv in range(2):
                    pstr = ps_tr.tile([128, 128], F32, tag="tr")
                    nc.tensor.transpose(
                        pstr[:], vt_dr[dv][:, cch * 128 : (cch + 1) * 128], ident[:]
                    )
                    nc.vector.tensor_copy(
                        vp[:, cch // 2, cch % 2, dv * 128 : (dv + 1) * 128],
                        pstr[:],
                    )
            vtr_done = upto

        for bi, (r0, nr) in enumerate(BLOCKS):
            L = (nr - 1) * 66 + 64
            for dkvt in range(4):
                ps = ps_mm.tile([128, 1024], F32, tag="mm")
                for si, (sh, sw) in enumerate(shifts):
                    base = (r0 + sh) * 66 + sw
                    nc.tensor.matmul(
                        ps[:, 0:L],
                        lhsT=wkv_sb[:, dkvt, si, :, :],
                        rhs=xpad[:, :, base : base + L],
                        start=(si == 0),
                        stop=(si == 8),
                        perf_mode=DR,
                    )
                src = ps[:, 0 : nr * 66].rearrange("p (r w) -> p r w", w=66)[
                    :, :, 0:64
                ]
                bias = smalls[:, 2 + dkvt : 3 + dkvt]
                if dkvt < 2:
                    dst = kt[:, dkvt, r0 * 64 : (r0 + nr) * 64].rearrange(
                        "p (r w) -> p r w", w=64
                    )
                else:
                    dst = vt_dr[dkvt - 2][:, r0 * 64 : (r0 + nr) * 64].rearrange(
                        "p (r w) -> p r w", w=64
                    )
                nc.scalar.activation(
                    dst, src, mybir.ActivationFunctionType.Relu,
                    bias=bias, scale=1.0,
                )
            for dqt in range(2):
                psq = ps_mm.tile([128, 1024], F32, tag="mm")
                base = (r0 + 1) * 66 + 1
                nc.tensor.matmul(
                    psq[:, 0:L],
                    lhsT=wq_sb[:, :, dqt * 128 : (dqt + 1) * 128],
                    rhs=xpad[:, :, base : base + L],
                    start=True,
                    stop=True,
                    perf_mode=DR,
                )
                srcq = psq[:, 0 : nr * 66].rearrange("p (r w) -> p r w", w=66)[
                    :, :, 0:64
                ]
                dstq = qt[:, dqt, r0 * 64 : (r0 + nr) * 64].rearrange(
                    "p (r w) -> p r w", w=64
                )
                nc.scalar.activation(
                    dstq, srcq, mybir.ActivationFunctionType.Relu,
                    bias=smalls[:, dqt : dqt + 1], scale=1.0,
                )
            if bi == 0:
                # prime the Exp ACT table while the PE is busy with conv
                pexp = tiny.tile([128, 1], F32, tag="pexp")
                nc.scalar.activation(
                    pexp[:], smalls[:, 0:1],
                    mybir.ActivationFunctionType.Exp, bias=0.0, scale=0.0,
                )
            emit_v_transposes((r0 + nr) // 2)

        # ---- phase C: attention, S(i) interleaved with A@V(i-1) ----
        def emit_av_epilogue(i, it, psa):
            zrec = tiny.tile([128, 1], F32, tag="zrec")
            nc.vector.reciprocal(zrec[:], psa[:, 256:257])
            rn = rn_pool.tile([128, C], F32)
            nc.vector.tensor_scalar_mul(rn[:], psa[:, 0:256], zrec[:])
            col = i * IBLK + it * 128
            scol = 4 * i + it
            for d in range(2):
                pstr = ps_tr.tile([128, 128], F32, tag="tr")
                nc.tensor.transpose(
                    pstr[:], rn[:, d * 128 : (d + 1) * 128], ident[:]
                )
                nc.vector.scalar_tensor_tensor(
                    out=y[d][:, col : col + 128],
                    in0=pstr[:],
                    scalar=1.0,
                    in1=xres[d][:, col : col + 128],
                    op0=mybir.AluOpType.mult,
                    op1=mybir.AluOpType.add,
                    accum_out=ssum[d][:, scol : scol + 1],
                )
                sq_t = sq_pool.tile([128, 128], F32)
                nc.vector.scalar_tensor_tensor(
                    out=sq_t[:],
                    in0=y[d][:, col : col + 128],
                    scalar=1.0,
                    in1=y[d][:, col : col + 128],
                    op0=mybir.AluOpType.mult,
                    op1=mybir.AluOpType.mult,
                    accum_out=ssq[d][:, scol : scol + 1],
                )

        NT2 = N_JT // 2  # 16 j-pair tiles per A@V accumulation chain

        def emit_av_steps(i, lo, hi):
            for k in range(lo, hi):
                it, t = divmod(k, NT2)
                if t == 0:
                    av_psa[i] = ps_av.tile(
                        [128, 257], F32, tag="av", name=f"psa_{i}_{it}"
                    )
                nc.tensor.matmul(
                    av_psa[i][:],
                    lhsT=et_tiles[i][:, 2 * t : 2 * t + 2,
                                     it * 128 : (it + 1) * 128],
                    rhs=vp[:, t, :, 0:257],
                    start=(t == 0),
                    stop=(t == NT2 - 1),
                    perf_mode=DR,
                )
                if t == NT2 - 1:
                    emit_av_epilogue(i, it, av_psa[i])

        partA = const.tile([128, 4], F32)
        for i in range(N_IBLK + 1):
            if i == N_IBLK:
                # pre-reduce blocks 0-6's BN partials while the PE drains
                # the final block's A@V chain
                for d in range(2):
                    nc.vector.tensor_reduce(
                        partA[:, d : d + 1],
                        ssum[d][:, 0:28],
                        axis=mybir.AxisListType.X,
                        op=mybir.AluOpType.add,
                    )
                    nc.vector.tensor_reduce(
                        partA[:, 2 + d : 3 + d],
                        ssq[d][:, 0:28],
                        axis=mybir.AxisListType.X,
                        op=mybir.AluOpType.add,
                    )
            if i < N_IBLK:
                et_tiles[i] = et_pool.tile(
                    [128, N_JT, IBLK], FP8, tag="et", name=f"et_{i}"
                )
            for jp in range(16):
                if i < N_IBLK:
                    emit_s_group(i, jp)
                if i > 0:
                    emit_av_steps(i - 1, jp * 4, jp * 4 + 4)

        # ---- phase D: BN stats all-reduce + normalize + writeout ----
        prime = tiny.tile([128, 1], F32, tag="prime")
        nc.scalar.activation(
            prime[:], smalls[:, 0:1], mybir.ActivationFunctionType.Identity,
            bias=0.0, scale=1.0,
        )
        eps_t = tiny.tile([128, 1], F32, tag="eps")
        nc.vector.memset(eps_t[:], BN_EPS)
        prime2 = tiny.tile([128, 1], F32, tag="prime2")
        nc.scalar.activation(
            prime2[:], smalls[:, 6:7], mybir.ActivationFunctionType.Sqrt,
            bias=eps_t[:], scale=1.0,
        )
        # partA (blocks 0-6) was pre-reduced during the A@V(7) drain; only
        # block 7's four columns remain on the critical path here
        partB = const.tile([128, 4], F32)
        partial = const.tile([128, 4], F32)  # [sum0, sum1, sq0, sq1]
        for d in range(2):
            nc.vector.tensor_reduce(
                partB[:, d : d + 1],
                ssum[d][:, 28:32],
                axis=mybir.AxisListType.X,
                op=mybir.AluOpType.add,
            )
            nc.vector.tensor_reduce(
                partB[:, 2 + d : 3 + d],
                ssq[d][:, 28:32],
                axis=mybir.AxisListType.X,
                op=mybir.AluOpType.add,
            )
        nc.vector.tensor_add(partial[:], partA[:], partB[:])
        inb = dram.tile([128, 4], F32)
        outb = dram.tile([128, 4], F32)
        nc.sync.dma_start(out=inb[:], in_=partial[:])
        nc.gpsimd.collective_compute(
            "AllReduce",
            mybir.AluOpType.add,
            replica_groups=replica_groups,
            ins=[inb.opt()],
            outs=[outb.opt()],
        )
        g = const.tile([128, 4], F32)
        nc.sync.dma_start(out=g[:], in_=outb[:])

        inv_n = 1.0 / float(len(replica_groups[0]) * N)
        ab = const.tile([128, 4], F32)  # [a0, a1, b0, b1]
        mean2 = tiny.tile([128, 2], F32, tag="mean2")
        msq2 = tiny.tile([128, 2], F32, tag="msq2")
        nc.vector.tensor_scalar_mul(mean2[:], g[:, 0:2], inv_n)
        nc.vector.tensor_scalar_mul(msq2[:], g[:, 2:4], inv_n)
        var2 = tiny.tile([128, 2], F32, tag="var2")
        nc.vector.tensor_mul(var2[:], mean2[:], mean2[:])
        nc.vector.tensor_sub(var2[:], msq2[:], var2[:])
        std2 = tiny.tile([128, 2], F32, tag="std2")
        nc.scalar.activation(
            std2[:], var2[:], mybir.ActivationFunctionType.Sqrt,
            bias=eps_t[:], scale=1.0,
        )
        rstd2 = tiny.tile([128, 2], F32, tag="rstd2")
        nc.vector.reciprocal(rstd2[:], std2[:])
        nc.vector.tensor_mul(ab[:, 0:2], rstd2[:], smalls[:, 6:8])
        t2 = tiny.tile([128, 2], F32, tag="t2")
        nc.vector.tensor_mul(t2[:], mean2[:], ab[:, 0:2])
        nc.vector.tensor_sub(ab[:, 2:4], smalls[:, 8:10], t2[:])

        CH = 512
        for k in range(N // CH):
            sl = slice(k * CH, (k + 1) * CH)
            for d in range(2):
                a_v = ab[:, d : d + 1]
                b_v = ab[:, 2 + d : 3 + d]
                cs = slice(d * 128, (d + 1) * 128)
                o_t = sq_pool.tile([128, CH], BF16, tag="stg", bufs=8)
                # ScalarE chunks run 813ns vs DVE 545ns: give ScalarE 6 of
                # the 16 chunks so both engines finish together
                if (2 * k + d) % 8 < 3:
                    nc.scalar.activation(
                        o_t[:],
                        y[d][:, sl],
                        mybir.ActivationFunctionType.Identity,
                        bias=b_v,
                        scale=a_v,
                    )
                else:
                    nc.vector.tensor_scalar(
                        out=o_t[:],
                        in0=y[d][:, sl],
                        scalar1=a_v,
                        scalar2=b_v,
                        op0=mybir.AluOpType.mult,
                        op1=mybir.AluOpType.add,
                    )
                # spread the writeout across three DMA paths (both HWDGE
                # queue groups + gpsimd SWDGE) so the chunks drain in
                # parallel
                eng = (nc.scalar, nc.sync, nc.gpsimd)[(2 * k + d) % 3]
                eng.dma_start(out=out_d[cs, sl], in_=o_t[:])


def pack_inputs(x, wq, bq, wkv, bkv, gamma, beta):
    """Host-side packing: per-core input maps (fp8 DoubleRow layouts)."""
    import ml_dtypes

    f8 = ml_dtypes.float8_e4m3
    B = x.shape[0]
    xc = np.ascontiguousarray(x.reshape(B, C, HW, HW).astype(np.float32))
    # xpad [B, 128, 2, FLATP] fp8: channel c = p + 128*s, flat padded 66x66
    xp = np.zeros((B, 2, 128, PW, PW), np.float32)
    xp[:, 0, :, 1:-1, 1:-1] = xc[:, 0:128]
    xp[:, 1, :, 1:-1, 1:-1] = xc[:, 128:256]
    xpad = np.zeros((B, 128, 2, FLATP), f8)
    xpad[:, :, :, 0:FLAT] = np.clip(
        xp.transpose(0, 2, 1, 3, 4).reshape(B, 128, 2, FLAT), -240, 240
    ).astype(f8)
    # wkv [128, 4, 9, 2, 128]: [p, o//128, kh*3+kw, s, o%128] =
    # wkv[o, p+128s, kh, kw]; the o-group as dim1 keeps each group's DMA
    # contiguous per partition so group 0 can land first
    wkvf = np.clip(wkv.astype(np.float32), -240, 240)
    wkv_dr = np.ascontiguousarray(
        wkvf.reshape(4, 128, 2, 128, 3, 3).transpose(3, 0, 4, 5, 2, 1).reshape(
            128, 4, 9, 2, 128
        )
    ).astype(f8)
    # wq [128, 2, 256]: [p, s, o] = wq[o, p+128s]
    wqf = np.clip(wq.reshape(C, C).astype(np.float32), -240, 240)
    wq_dr = np.ascontiguousarray(
        wqf.reshape(C, 2, 128).transpose(2, 1, 0)
    ).astype(f8)
    xresb = np.ascontiguousarray(xc.reshape(B, C, N)).astype(ml_dtypes.bfloat16)
    smalls = np.zeros((128, 10), np.float32)
    smalls[:, 0] = bq[0:128]
    smalls[:, 1] = bq[128:256]
    for k in range(4):
        smalls[:, 2 + k] = bkv[k * 128 : (k + 1) * 128]
    smalls[:, 6] = gamma[0:128]
    smalls[:, 7] = gamma[128:256]
    smalls[:, 8] = beta[0:128]
    smalls[:, 9] = beta[128:256]
    return [
        {
            "xpad": xpad[b],
            "xres": xresb[b],
            "wkv": wkv_dr,
            "wq": wq_dr,
            "smalls": smalls,
        }
        for b in range(B)
    ]


_CACHED = {}


def get_program():
    if "nc" not in _CACHED:
        _CACHED["nc"] = build_program()
    return _CACHED["nc"]


def kernel(x, wq, bq, wkv, bkv, gamma, beta, trace=False):
    x = np.asarray(x)
    in_maps = pack_inputs(
        x,
        np.asarray(wq),
        np.asarray(bq),
        np.asarray(wkv),
        np.asarray(bkv),
        np.asarray(gamma),
        np.asarray(beta),
    )
    nc = get_program()
    try:
        res = run_bass_kernel_spmd(
            nc, in_maps, core_ids=list(range(N_CORES)), trace=trace
        )
    except Exception:
        # a wedged axon terminal (LoadExecutable/exec errors) is recoverable
        import ctypes

        try:
            lib = ctypes.CDLL("/opt/axon/libaxon_pjrt.so")
            lib.axon_reset.restype = ctypes.c_int64
            lib.axon_reset()
        except Exception:
            pass
        res = run_bass_kernel_spmd(
            nc, in_maps, core_ids=list(range(N_CORES)), trace=trace
        )
    out = np.stack(
        [
            res.results[b]["out"].astype(np.float32).reshape(C, HW, HW)
            for b in range(N_CORES)
        ]
    )
    if trace:
        kernel.last_results = res
    return out
